# revision 1
# baseline (speedup 1.0000x reference)
"""DarcyFlow operator Ax = conv2x2(4ch a-weighted shifts of x) zero-padded.

Strategy (8 NeuronCores, data-parallel over image ROWS):
  - Core c owns output rows [128c .. 128c+127] of all 16 images. This keeps
    the replicated coefficient field `a` tiny per core (~1MB vs 8.4MB for
    batch-parallel) and lets the a-derived tiles be loaded once per core and
    reused across all 16 images.
  - The operator is decomposed into 4 elementwise products
        Q4 = a[r]   * x[r],   Q3 = a[r]   * x[r, +1col]
        Q1 = a[r-1] * x[r],   Q2 = a[r-1] * x[r, +1col]
    (computed on the Vector engine; fp32 tensor_tensor is the bottleneck at
    1 elem/cycle/lane) followed by the 16 conv taps accumulated on the
    otherwise-idle Tensor engine as 8 banded float32r matmuls into PSUM
    (row shifts live in the banded stationary matrix, column shifts in the
    moving-operand access pattern). ScalarE drains PSUM -> SBUF.
  - Per image one 128-row window produces 126 output rows; the remaining
    2 rows/image are computed by one packed tail window (16 img x 4 rows).
  - Borders: output DRAM starts zeroed; stores skip border cols; the host
    drops the one garbage row computed at the global top/bottom edge.
"""

import numpy as np

B = 16
N = 1024
NCORES = 8
SLAB = N // NCORES  # 128
WX = N + 2          # padded x width (zero col both sides)
WQ = N + 1          # product width
COLT = 512          # psum bank column tile

_K = np.array(
    [
        [[-1 / 6, 2 / 3], [-1 / 3, -1 / 6]],  # K1 (ch Q1)
        [[2 / 3, -1 / 6], [-1 / 6, -1 / 3]],  # K2 (ch Q2)
        [[-1 / 6, -1 / 3], [2 / 3, -1 / 6]],  # K3 (ch Q3)
        [[-1 / 3, -1 / 6], [-1 / 6, 2 / 3]],  # K4 (ch Q4)
    ],
    dtype=np.float32,
)

# pass order: (channel, dj). channel 0..3 <-> Q1,Q2,Q3,Q4
PASS_DEFS = [(ch, dj) for dj in (0, 1) for ch in (0, 1, 2, 3)]


def _build_weights():
    """Host-built banded lhsT matrices for the 8 main + 8 tail passes."""
    wm = np.zeros((8, SLAB, SLAB), dtype=np.float32)
    wt = np.zeros((8, 64, 32), dtype=np.float32)
    for p, (ch, dj) in enumerate(PASS_DEFS):
        off = 0 if ch < 2 else -1  # Q1/Q2 band k-m in {0,1}; Q3/Q4 in {-1,0}
        for m in range(1, SLAB - 1):
            for di in range(2):
                wm[p, m + off + di, m] = _K[ch, di, dj]
        for b in range(16):
            for u in range(2):
                for di in range(2):
                    t = u + di + (1 if ch < 2 else 0)
                    wt[p, 4 * b + t, 2 * b + u] = _K[ch, di, dj]
    return (
        np.ascontiguousarray(wm.transpose(1, 0, 2).reshape(SLAB, 8 * SLAB)),
        np.ascontiguousarray(wt.transpose(1, 0, 2).reshape(64, 8 * 32)),
    )


def _shard_inputs(x, a):
    """Per-core padded input arrays. x: [B,1,N,N], a: [1,1,N-1,N-1]."""
    x = np.asarray(x, dtype=np.float32).reshape(B, N, N)
    a = np.asarray(a, dtype=np.float32).reshape(N - 1, N - 1)

    # zero-padded a lookup: arow(r) valid for r in [0, N-2]
    apad = np.zeros((N + 2, WQ), dtype=np.float32)
    apad[1 : N, 1 : N] = a  # apad[r+1, 1:N] = a[r]

    def arow(r):  # global a row r, padded to width WQ
        return apad[r + 1]

    wm, wt = _build_weights()
    shards = []
    for c in range(NCORES):
        r0 = c * SLAB
        xc = np.zeros((B, SLAB + 2, WX), dtype=np.float32)
        lo = max(0, r0 - 1)
        hi = min(N, r0 + SLAB + 1)
        xc[:, lo - (r0 - 1) : hi - (r0 - 1), 1 : N + 1] = x[:, lo:hi, :]

        a0m = np.stack([arow(r0 - 1 + k) for k in range(SLAB)])
        a1m = np.stack([arow(r0 - 2 + k) for k in range(SLAB)])
        a0t = np.stack([arow(r0 + 125 + t) for _ in range(16) for t in range(4)])
        a1t = np.stack([arow(r0 + 124 + t) for _ in range(16) for t in range(4)])
        shards.append(
            {
                "xc": xc,
                "xt": np.ascontiguousarray(xc[:, SLAB - 2 : SLAB + 2, :].reshape(64, WX)),
                "a01m": np.ascontiguousarray(np.hstack([a0m, a1m])),
                "a01t": np.ascontiguousarray(np.hstack([a0t, a1t])),
                "wm": wm,
                "wt": wt,
            }
        )
    return shards


_CACHE = {}


def _build_module(iters=1, variant="full"):
    """Build + compile the (identical-program) per-core Bass module.

    iters > 1 wraps the compute in a hardware For loop (for benchmarking
    steady-state per-iteration time via wall-clock deltas).
    variant: "full" | "dma" (loads only) | "dve" (loads+products) |
             "nodve" (loads+matmuls+stores, skip products) — timing probes.
    """
    key = ("nc", iters, variant)
    if key in _CACHE:
        return _CACHE[key]

    import concourse.bacc as bacc
    import concourse.tile as tile
    from concourse import mybir

    f32 = mybir.dt.float32
    f32r = mybir.dt.float32r

    nc = bacc.Bacc("TRN2", target_bir_lowering=False, debug=False,
                   num_devices=NCORES)

    xc_d = nc.dram_tensor("xc", [B, SLAB + 2, WX], f32, kind="ExternalInput").ap()
    xt_d = nc.dram_tensor("xt", [64, WX], f32, kind="ExternalInput").ap()
    a01m_d = nc.dram_tensor("a01m", [SLAB, 2 * WQ], f32, kind="ExternalInput").ap()
    a01t_d = nc.dram_tensor("a01t", [64, 2 * WQ], f32, kind="ExternalInput").ap()
    wm_d = nc.dram_tensor("wm", [SLAB, 8 * SLAB], f32r, kind="ExternalInput").ap()
    wt_d = nc.dram_tensor("wt", [64, 8 * 32], f32r, kind="ExternalInput").ap()
    out_d = nc.dram_tensor("out", [B, SLAB, N], f32, kind="ExternalOutput").ap()
    outt_d = nc.dram_tensor("outt", [32, N], f32, kind="ExternalOutput").ap()

    with tile.TileContext(nc) as tc:
        with (
            tc.tile_pool(name="const", bufs=1) as const,
            tc.tile_pool(name="xin", bufs=4) as xin,
            tc.tile_pool(name="prod", bufs=3) as prod,
            tc.tile_pool(name="stage", bufs=4) as stage,
            tc.tile_pool(name="psum", bufs=6, space="PSUM") as psum,
        ):
            # window-0-gating constants first (a01t/wt only gate the tail)
            A01m = const.tile([SLAB, 2 * WQ], f32)
            nc.gpsimd.dma_start(A01m[:], a01m_d[:])
            Wm = const.tile([SLAB, 8 * SLAB], f32r)
            nc.scalar.dma_start(Wm[:], wm_d[:])
            A01t = const.tile([64, 2 * WQ], f32)
            nc.gpsimd.dma_start(A01t[:], a01t_d[:])
            Wt = const.tile([64, 8 * 32], f32r)
            nc.scalar.dma_start(Wt[:], wt_d[:])

            def window(X, A01, P, M, wtile, wstride, st, ps_bufs):
                """One banded-stencil window.
                X: [P, WX] input tile, A01: [P, 2*WQ] = [A0 | A1],
                M: out partitions, wtile: weights, st: staging tile.
                """
                if variant == "dma":
                    return
                # q41 = [A0*X | A1*X], q32 = [A0*Xs | A1*Xs]  (one DVE op each)
                q41 = prod.tile([P, 2 * WQ], f32r, name=f"q41_{P}", tag=f"q41_{P}")
                q32 = prod.tile([P, 2 * WQ], f32r, name=f"q32_{P}", tag=f"q32_{P}")
                if variant == "nodve":
                    # touch one column so the tiles are allocated (timing probe)
                    nc.vector.tensor_scalar_mul(q41[:, 0:1], X[:, 0:1], 1.0)
                    nc.vector.tensor_scalar_mul(q32[:, 0:1], X[:, 0:1], 1.0)
                if variant in ("full", "dve"):
                    nc.vector.tensor_mul(
                        q41[:].rearrange("p (c w) -> p c w", c=2),
                        A01[:].rearrange("p (c w) -> p c w", c=2),
                        X[:, 0:WQ][:, None, :].broadcast_to([P, 2, WQ]),
                    )
                    nc.vector.tensor_mul(
                        q32[:].rearrange("p (c w) -> p c w", c=2),
                        A01[:].rearrange("p (c w) -> p c w", c=2),
                        X[:, 1 : WQ + 1][:, None, :].broadcast_to([P, 2, WQ]),
                    )
                if variant in ("dma", "dve"):
                    return
                # channel views: Q1=A1*X, Q2=A1*Xs, Q3=A0*Xs, Q4=A0*X
                qoff = [(q41, WQ), (q32, WQ), (q32, 0), (q41, 0)]
                for t in range(2):
                    ps = psum.tile([M, COLT], f32, name=f"ps_{P}", tag=f"ps_{P}",
                                   bufs=ps_bufs)
                    for p, (ch, dj) in enumerate(PASS_DEFS):
                        q, off = qoff[ch]
                        nc.tensor.matmul(
                            ps[:],
                            wtile[:, p * wstride : (p + 1) * wstride],
                            q[:, off + t * COLT + dj : off + t * COLT + dj + COLT],
                            start=(p == 0),
                            stop=(p == 7),
                        )
                    nc.scalar.copy(st[:, t * COLT : (t + 1) * COLT], ps[:])

            def body():
                # 16 main windows (one per image)
                for b in range(B):
                    X = xin.tile([SLAB, WX], f32, name="xw", tag="xw")
                    nc.sync.dma_start(X[:], xc_d[b, 0:SLAB, :])
                    st = stage.tile([SLAB, N], f32, name="stm", tag="stm")
                    window(X, A01m, SLAB, SLAB, Wm, SLAB, st, 6)
                    if variant == "full" or variant == "nodve":
                        nc.sync.dma_start(out_d[b, 0 : SLAB - 2, 1 : N - 1],
                                          st[1 : SLAB - 1, 1 : N - 1])

                # packed tail: 16 images x rows 126..129 -> out rows 126,127
                Xt = xin.tile([64, WX], f32, name="xtw", tag="xtw")
                nc.sync.dma_start(Xt[:], xt_d[:])
                stt = stage.tile([32, N], f32, name="stt", tag="stt")
                window(Xt, A01t, 64, 32, Wt, 32, stt, 2)
                if variant == "full" or variant == "nodve":
                    nc.sync.dma_start(outt_d[:, 1 : N - 1], stt[:, 1 : N - 1])

            if iters == 1:
                body()
            else:
                with tc.For_i(0, iters, 1):
                    body()

    nc.compile()
    _CACHE[key] = nc
    return nc


def run(inputs, trace=False, trace_kwargs=None, iters=1, variant="full"):
    """Run the sharded kernel; returns (full_output, BassKernelResults)."""
    from concourse.bass_utils import run_bass_kernel_spmd

    nc = _build_module(iters, variant)
    in_maps = _shard_inputs(inputs["x"], inputs["a"])
    res = run_bass_kernel_spmd(
        nc,
        in_maps,
        core_ids=list(range(NCORES)),
        trace=trace,
        **(trace_kwargs or {}),
    )
    full = np.zeros((B, 1, N, N), dtype=np.float32)
    for c in range(NCORES):
        oc = np.array(res.results[c]["out"])  # [B, SLAB, N]
        oc[:, SLAB - 2 : SLAB, :] = res.results[c]["outt"].reshape(B, 2, N)
        r0 = c * SLAB
        lo = 1 if c == 0 else 0            # drop garbage global row 0
        hi = SLAB - 1 if c == NCORES - 1 else SLAB  # drop garbage row N-1
        full[:, 0, r0 + lo : r0 + hi, 1 : N - 1] = oc[:, lo:hi, 1 : N - 1]
    return full, res


def kernel(**inputs) -> np.ndarray:
    out, _ = run(inputs, trace=False)
    return out



# revision 2
# speedup vs baseline: 1.2653x; 1.2653x over previous
"""DarcyFlow operator Ax = conv2x2(4ch a-weighted shifts of x) zero-padded.

Strategy (8 NeuronCores, data-parallel over image ROWS):
  - Core c owns output rows [128c .. 128c+127] of all 16 images. This keeps
    the replicated coefficient field `a` tiny per core (~0.5MB) and lets the
    a-derived tiles be loaded once per core and reused across all 16 images.
  - The operator is decomposed into 4 elementwise products
        Q4 = a[r]   * x[r],   Q3 = a[r]   * x[r, +1col]
        Q1 = a[r-1] * x[r],   Q2 = a[r-1] * x[r, +1col]
    (computed on the Vector engine in fp16 — 2-byte dtypes get the DVE
    2x_1p fast path) followed by the 16 conv taps accumulated on the
    Tensor engine as 8 banded fp16 matmuls into fp32 PSUM (row shifts live
    in the banded stationary matrix, column shifts in the moving-operand
    access pattern). fp16 weights also enable Fast Weight Load. ScalarE
    drains PSUM -> SBUF (fp16).
  - Per image one 128-row window produces 126 output rows; the remaining
    2 rows/image are computed by one packed tail window (16 img x 4 rows).
  - Borders: stores skip border cols; the host drops the one garbage row
    computed at the global top/bottom edge and zero-fills borders.

All device compute/IO is fp16 except PSUM accumulation (fp32). Max rel
error vs the fp64 reference is ~2e-3, well inside the 2e-2 gate.
"""

import numpy as np

B = 16
N = 1024
NCORES = 8
SLAB = N // NCORES  # 128
WX = N + 2          # padded x width (zero col both sides)
WQ = N + 1          # product width
COLT = 512          # psum bank column tile

_K = np.array(
    [
        [[-1 / 6, 2 / 3], [-1 / 3, -1 / 6]],  # K1 (ch Q1)
        [[2 / 3, -1 / 6], [-1 / 6, -1 / 3]],  # K2 (ch Q2)
        [[-1 / 6, -1 / 3], [2 / 3, -1 / 6]],  # K3 (ch Q3)
        [[-1 / 3, -1 / 6], [-1 / 6, 2 / 3]],  # K4 (ch Q4)
    ],
    dtype=np.float32,
)

# pass order: (channel, dj). channel 0..3 <-> Q1,Q2,Q3,Q4
PASS_DEFS = [(ch, dj) for dj in (0, 1) for ch in (0, 1, 2, 3)]


def _build_weights():
    """Host-built banded lhsT matrices for the 8 main + 8 tail passes."""
    wm = np.zeros((8, SLAB, SLAB), dtype=np.float32)
    wt = np.zeros((8, 64, 32), dtype=np.float32)
    for p, (ch, dj) in enumerate(PASS_DEFS):
        off = 0 if ch < 2 else -1  # Q1/Q2 band k-m in {0,1}; Q3/Q4 in {-1,0}
        for m in range(1, SLAB - 1):
            for di in range(2):
                wm[p, m + off + di, m] = _K[ch, di, dj]
        for b in range(16):
            for u in range(2):
                for di in range(2):
                    t = u + di + (1 if ch < 2 else 0)
                    wt[p, 4 * b + t, 2 * b + u] = _K[ch, di, dj]
    return (
        np.ascontiguousarray(
            wm.transpose(1, 0, 2).reshape(SLAB, 8 * SLAB).astype(np.float16)
        ),
        np.ascontiguousarray(
            wt.transpose(1, 0, 2).reshape(64, 8 * 32).astype(np.float16)
        ),
    )


def _shard_inputs(x, a):
    """Per-core padded fp16 input arrays. x: [B,1,N,N], a: [1,1,N-1,N-1]."""
    x = np.asarray(x, dtype=np.float32).reshape(B, N, N).astype(np.float16)
    a = np.asarray(a, dtype=np.float32).reshape(N - 1, N - 1).astype(np.float16)

    # zero-padded a lookup: arow(r) valid for r in [0, N-2]
    apad = np.zeros((N + 2, WQ), dtype=np.float16)
    apad[1 : N, 1 : N] = a  # apad[r+1, 1:N] = a[r]

    def arow(r):  # global a row r, padded to width WQ
        return apad[r + 1]

    wm, wt = _build_weights()
    shards = []
    for c in range(NCORES):
        r0 = c * SLAB
        xc = np.zeros((B, SLAB + 2, WX), dtype=np.float16)
        lo = max(0, r0 - 1)
        hi = min(N, r0 + SLAB + 1)
        xc[:, lo - (r0 - 1) : hi - (r0 - 1), 1 : N + 1] = x[:, lo:hi, :]

        a0m = np.stack([arow(r0 - 1 + k) for k in range(SLAB)])
        a1m = np.stack([arow(r0 - 2 + k) for k in range(SLAB)])
        a0t = np.stack([arow(r0 + 125 + t) for _ in range(16) for t in range(4)])
        a1t = np.stack([arow(r0 + 124 + t) for _ in range(16) for t in range(4)])
        shards.append(
            {
                "xc": xc,
                "xt": np.ascontiguousarray(xc[:, SLAB - 2 : SLAB + 2, :].reshape(64, WX)),
                "a01m": np.ascontiguousarray(np.hstack([a0m, a1m])),
                "a01t": np.ascontiguousarray(np.hstack([a0t, a1t])),
                "wm": wm,
                "wt": wt,
            }
        )
    return shards


_CACHE = {}


def _build_module(iters=1, variant="full"):
    """Build + compile the (identical-program) per-core Bass module.

    iters > 1 wraps the compute in a hardware For loop (for benchmarking
    steady-state per-iteration time via wall-clock deltas).
    variant: "full" | "dma" (loads only) | "dve" (loads+products) |
             "nodve" (loads+matmuls+stores, skip products) — timing probes.
    """
    key = ("nc", iters, variant)
    if key in _CACHE:
        return _CACHE[key]

    import concourse.bacc as bacc
    import concourse.tile as tile
    from concourse import mybir

    f16 = mybir.dt.float16
    f32 = mybir.dt.float32

    nc = bacc.Bacc("TRN2", target_bir_lowering=False, debug=False,
                   num_devices=NCORES)

    xc_d = nc.dram_tensor("xc", [B, SLAB + 2, WX], f16, kind="ExternalInput").ap()
    xt_d = nc.dram_tensor("xt", [64, WX], f16, kind="ExternalInput").ap()
    a01m_d = nc.dram_tensor("a01m", [SLAB, 2 * WQ], f16, kind="ExternalInput").ap()
    a01t_d = nc.dram_tensor("a01t", [64, 2 * WQ], f16, kind="ExternalInput").ap()
    wm_d = nc.dram_tensor("wm", [SLAB, 8 * SLAB], f16, kind="ExternalInput").ap()
    wt_d = nc.dram_tensor("wt", [64, 8 * 32], f16, kind="ExternalInput").ap()
    out_d = nc.dram_tensor("out", [B, SLAB, N], f16, kind="ExternalOutput").ap()
    outt_d = nc.dram_tensor("outt", [32, N], f16, kind="ExternalOutput").ap()

    with tile.TileContext(nc) as tc:
        with (
            tc.tile_pool(name="const", bufs=1) as const,
            tc.tile_pool(name="xin", bufs=4) as xin,
            tc.tile_pool(name="prod", bufs=3) as prod,
            tc.tile_pool(name="stage", bufs=4) as stage,
            tc.tile_pool(name="psum", bufs=6, space="PSUM") as psum,
        ):
            # window-0-gating constants first (a01t/wt only gate the tail)
            A01m = const.tile([SLAB, 2 * WQ], f16)
            nc.gpsimd.dma_start(A01m[:], a01m_d[:])
            Wm = const.tile([SLAB, 8 * SLAB], f16)
            nc.scalar.dma_start(Wm[:], wm_d[:])
            A01t = const.tile([64, 2 * WQ], f16)
            nc.gpsimd.dma_start(A01t[:], a01t_d[:])
            Wt = const.tile([64, 8 * 32], f16)
            nc.scalar.dma_start(Wt[:], wt_d[:])

            def window(X, A01, P, M, wtile, wstride, st, ps_bufs):
                """One banded-stencil window.
                X: [P, WX] input tile, A01: [P, 2*WQ] = [A0 | A1],
                M: out partitions, wtile: weights, st: staging tile.
                """
                if variant == "dma":
                    return
                # q41 = [A0*X | A1*X], q32 = [A0*Xs | A1*Xs]  (one DVE op each)
                q41 = prod.tile([P, 2 * WQ], f16, name=f"q41_{P}", tag=f"q41_{P}")
                q32 = prod.tile([P, 2 * WQ], f16, name=f"q32_{P}", tag=f"q32_{P}")
                if variant == "nodve":
                    # touch one column so the tiles are allocated (timing probe)
                    nc.vector.tensor_scalar_mul(q41[:, 0:1], X[:, 0:1], 1.0)
                    nc.vector.tensor_scalar_mul(q32[:, 0:1], X[:, 0:1], 1.0)
                if variant in ("full", "dve"):
                    nc.vector.tensor_mul(
                        q41[:].rearrange("p (c w) -> p c w", c=2),
                        A01[:].rearrange("p (c w) -> p c w", c=2),
                        X[:, 0:WQ][:, None, :].broadcast_to([P, 2, WQ]),
                    )
                    nc.vector.tensor_mul(
                        q32[:].rearrange("p (c w) -> p c w", c=2),
                        A01[:].rearrange("p (c w) -> p c w", c=2),
                        X[:, 1 : WQ + 1][:, None, :].broadcast_to([P, 2, WQ]),
                    )
                if variant in ("dma", "dve"):
                    return
                # channel views: Q1=A1*X, Q2=A1*Xs, Q3=A0*Xs, Q4=A0*X
                qoff = [(q41, WQ), (q32, WQ), (q32, 0), (q41, 0)]
                for t in range(2):
                    ps = psum.tile([M, COLT], f32, name=f"ps_{P}", tag=f"ps_{P}",
                                   bufs=ps_bufs)
                    for p, (ch, dj) in enumerate(PASS_DEFS):
                        q, off = qoff[ch]
                        nc.tensor.matmul(
                            ps[:],
                            wtile[:, p * wstride : (p + 1) * wstride],
                            q[:, off + t * COLT + dj : off + t * COLT + dj + COLT],
                            start=(p == 0),
                            stop=(p == 7),
                        )
                    nc.scalar.copy(st[:, t * COLT : (t + 1) * COLT], ps[:])

            def body():
                # 16 main windows (one per image)
                for b in range(B):
                    X = xin.tile([SLAB, WX], f16, name="xw", tag="xw")
                    nc.sync.dma_start(X[:], xc_d[b, 0:SLAB, :])
                    st = stage.tile([SLAB, N], f16, name="stm", tag="stm")
                    window(X, A01m, SLAB, SLAB, Wm, SLAB, st, 6)
                    if variant == "full" or variant == "nodve":
                        nc.sync.dma_start(out_d[b, 0 : SLAB - 2, 1 : N - 1],
                                          st[1 : SLAB - 1, 1 : N - 1])

                # packed tail: 16 images x rows 126..129 -> out rows 126,127
                Xt = xin.tile([64, WX], f16, name="xtw", tag="xtw")
                nc.sync.dma_start(Xt[:], xt_d[:])
                stt = stage.tile([32, N], f16, name="stt", tag="stt")
                window(Xt, A01t, 64, 32, Wt, 32, stt, 2)
                if variant == "full" or variant == "nodve":
                    nc.sync.dma_start(outt_d[:, 1 : N - 1], stt[:, 1 : N - 1])

            if iters == 1:
                body()
            else:
                with tc.For_i(0, iters, 1):
                    body()

    nc.compile()
    _CACHE[key] = nc
    return nc


def run(inputs, trace=False, trace_kwargs=None, iters=1, variant="full"):
    """Run the sharded kernel; returns (full_output, BassKernelResults)."""
    from concourse.bass_utils import run_bass_kernel_spmd

    nc = _build_module(iters, variant)
    in_maps = _shard_inputs(inputs["x"], inputs["a"])
    res = run_bass_kernel_spmd(
        nc,
        in_maps,
        core_ids=list(range(NCORES)),
        trace=trace,
        **(trace_kwargs or {}),
    )
    full = np.zeros((B, 1, N, N), dtype=np.float32)
    for c in range(NCORES):
        oc = np.array(res.results[c]["out"]).astype(np.float32)  # [B, SLAB, N]
        oc[:, SLAB - 2 : SLAB, :] = (
            np.array(res.results[c]["outt"]).astype(np.float32).reshape(B, 2, N)
        )
        r0 = c * SLAB
        lo = 1 if c == 0 else 0            # drop garbage global row 0
        hi = SLAB - 1 if c == NCORES - 1 else SLAB  # drop garbage row N-1
        full[:, 0, r0 + lo : r0 + hi, 1 : N - 1] = oc[:, lo:hi, 1 : N - 1]
    return full, res


def kernel(**inputs) -> np.ndarray:
    out, _ = run(inputs, trace=False)
    return out
